# revision 11
# baseline (speedup 1.0000x reference)
"""Trainium2 Bass kernel for nn_Attention (B=2, C=256, H=W=64, 8 heads).

Sharding: 8 cores = 2 batches x 4 query-chunks (1024 queries each), no
collectives. Each core gets its batch's full x with token columns rolled so
its own query chunk sits at columns 0:1024 (attention is permutation-
invariant over keys); it computes LN + projections + attention for its
queries and writes a [256, 1024] slice of the output.

Everything stays in the transposed [channel, token] layout (x's native DRAM
layout): LN (stats via ones-matmul over the partition dim) -> qT/kT
projections -> S^T = K Q^T (K=32 matmuls packed 4-way into PE row groups)
-> exp -> P^T V via stationary-V matmuls with an appended ones column
(softmax denominators for free) -> normalize -> out-projection -> residual.

exp is split between ScalarE (true exp) and VectorE (Schraudolph: Wq is
pre-scaled so PSUM = 128*log2(e)*logit; adding a magic bias and converting
f32->int16 yields the bf16 bit pattern of 2^y, ~3% max rel err, harmless
here because the attention branch contributes ~0.2% of the output next to
the residual).
"""

import numpy as np

B, C, H, W = 2, 256, 64, 64
N = H * W            # 4096 tokens
NH, HD = 8, 32       # heads, head_dim
NQ = N // 4          # queries per core
LN_EPS = 1e-5
LOG2E = 1.4426950408889634
LN2 = 0.6931471805599453
ATTN_SCALE = HD ** -0.5
A_SCALE = 128.0 * LOG2E * ATTN_SCALE   # folded into Wq on host
B16F = 16256.0 - 5.6                   # Schraudolph bias (calibrated)
ACT_EXP_SHARE = 5                      # j%8 < ACT_EXP_SHARE -> ScalarE exp

_PROFILE = False
_CACHE = {}


def _build():
    from concourse import bacc
    from concourse import mybir
    import concourse.tile as tile
    import dataclasses

    f32 = mybir.dt.float32
    bf16 = mybir.dt.bfloat16
    i16 = mybir.dt.int16
    ALU = mybir.AluOpType
    ACTF = mybir.ActivationFunctionType

    nc = bacc.Bacc("TRN2", target_bir_lowering=False)
    xd = nc.dram_tensor("x", [C, N], f32, kind="ExternalInput")
    wq = nc.dram_tensor("wqT", [C, C], bf16, kind="ExternalInput")  # pre-scaled
    wk = nc.dram_tensor("wkT", [C, C], bf16, kind="ExternalInput")
    wv = nc.dram_tensor("wvT", [C, C], bf16, kind="ExternalInput")
    wp = nc.dram_tensor("wpT", [C, C], bf16, kind="ExternalInput")
    gam = nc.dram_tensor("gam", [C, 1], f32, kind="ExternalInput")
    bet = nc.dram_tensor("bet", [C, 1], f32, kind="ExternalInput")
    bpd = nc.dram_tensor("bp", [C, 1], f32, kind="ExternalInput")
    od = nc.dram_tensor("out", [C, NQ], f32, kind="ExternalOutput")

    def bcast(ap, parts):
        # replicate one partition across `parts` partitions (DMA source only)
        return dataclasses.replace(ap, ap=[[0, parts]] + list(ap.ap[1:]))

    with tile.TileContext(nc) as tc:
        with tc.tile_pool(name="big", bufs=1) as big, \
             tc.tile_pool(name="sml", bufs=4) as sml:

            # ---- load inputs ----
            x_sb = [big.tile([128, N], f32, tag=f"x{c}", name=f"x{c}") for c in range(2)]
            for c in range(2):
                nc.sync.dma_start(out=x_sb[c][:, :], in_=xd[c * 128:(c + 1) * 128, :])
            w_sb = {}
            for name, t in (("q", wq), ("k", wk), ("v", wv), ("p", wp)):
                for c in range(2):
                    s = big.tile([128, C], bf16, tag=f"w{name}{c}", name=f"w{name}{c}")
                    nc.sync.dma_start(out=s[:, :], in_=t[c * 128:(c + 1) * 128, :])
                    w_sb[name, c] = s
            gam_sb = [big.tile([128, 1], f32, tag=f"g{c}", name=f"g{c}") for c in range(2)]
            bet_sb = [big.tile([128, 1], f32, tag=f"b{c}", name=f"b{c}") for c in range(2)]
            bp_sb = [big.tile([128, 1], f32, tag=f"bp{c}", name=f"bp{c}") for c in range(2)]
            for c in range(2):
                nc.sync.dma_start(out=gam_sb[c][:, :], in_=gam[c * 128:(c + 1) * 128, :])
                nc.sync.dma_start(out=bet_sb[c][:, :], in_=bet[c * 128:(c + 1) * 128, :])
                nc.sync.dma_start(out=bp_sb[c][:, :], in_=bpd[c * 128:(c + 1) * 128, :])
            ones_sb = big.tile([128, 1], f32, tag="ones", name="ones")
            nc.vector.memset(ones_sb[:, :], 1.0 / C)
            ones_row = big.tile([1, 128], f32, tag="onesr", name="onesr")
            nc.vector.memset(ones_row[:, :], 1.0)

            tn = [big.tile([128, N], bf16, tag=f"tn{c}", name=f"tn{c}") for c in range(2)]

            # ---- LayerNorm ----
            with tc.tile_pool(name="lnp", bufs=1) as lnp, \
                 tc.tile_pool(name="lns", bufs=2, space="PSUM") as lns:
                sq = [lnp.tile([128, N], f32, tag=f"sq{c}", name=f"sq{c}") for c in range(2)]
                for c in range(2):
                    nc.scalar.activation(sq[c][:, :], x_sb[c][:, :], ACTF.Square)
                eps_sc = lnp.tile([1, 1], f32, tag="epssc", name="epssc")
                nc.vector.memset(eps_sc[:, :], LN_EPS)
                for f in range(8):
                    sl = slice(f * 512, (f + 1) * 512)
                    mps = lns.tile([1, 512], f32, tag="mps", name="mps")
                    nc.tensor.matmul(mps[:, :], ones_sb[:, :], x_sb[0][:, sl], start=True, stop=False)
                    nc.tensor.matmul(mps[:, :], ones_sb[:, :], x_sb[1][:, sl], start=False, stop=True)
                    mu_sb = sml.tile([1, 512], f32, tag="musb", name="musb")
                    nc.vector.tensor_copy(mu_sb[:, :], mps[:, :])
                    eps_t = lns.tile([1, 512], f32, tag="eps", name="eps")
                    nc.tensor.matmul(eps_t[:, :], ones_sb[:, :], sq[0][:, sl], start=True, stop=False)
                    nc.tensor.matmul(eps_t[:, :], ones_sb[:, :], sq[1][:, sl], start=False, stop=True)
                    var_sb = sml.tile([1, 512], f32, tag="varsb", name="varsb")
                    nc.vector.tensor_tensor(var_sb[:, :], mu_sb[:, :], mu_sb[:, :], ALU.mult)
                    nc.vector.tensor_tensor(var_sb[:, :], eps_t[:, :], var_sb[:, :], ALU.subtract)
                    std_sb = sml.tile([1, 512], f32, tag="stdsb", name="stdsb")
                    nc.scalar.activation(std_sb[:, :], var_sb[:, :], ACTF.Sqrt, bias=eps_sc[:, :])
                    rs_sb = sml.tile([1, 512], f32, tag="rssb", name="rssb")
                    nc.vector.reciprocal(rs_sb[:, :], std_sb[:, :])
                    mu_b = lns.tile([128, 512], f32, tag="mub", name="mub")
                    rs_b = lns.tile([128, 512], f32, tag="rsb", name="rsb")
                    nc.tensor.matmul(mu_b[:, :], ones_row[:, :], mu_sb[:, :],
                                     start=True, stop=True, tile_position=(0, 0))
                    nc.tensor.matmul(rs_b[:, :], ones_row[:, :], rs_sb[:, :],
                                     start=True, stop=True, tile_position=(0, 0))
                    for c in range(2):
                        t = lnp.tile([128, 512], f32, tag=f"t{c}", name=f"t{c}")
                        nc.vector.tensor_tensor(t[:, :], x_sb[c][:, sl], mu_b[:, :], ALU.subtract)
                        nc.vector.tensor_tensor(t[:, :], t[:, :], rs_b[:, :], ALU.mult)
                        nc.vector.tensor_scalar(tn[c][:, sl], t[:, :], gam_sb[c][:, :],
                                                bet_sb[c][:, :], ALU.mult, ALU.add)

            # ---- q/k/v projections ----
            qT = [big.tile([128, NQ], bf16, tag=f"qT{c}", name=f"qT{c}") for c in range(2)]
            kT = [big.tile([128, N], bf16, tag=f"kT{c}", name=f"kT{c}") for c in range(2)]
            v_sb = big.tile([128, 32, NH, 33], bf16, tag="v", name="v")
            nc.vector.memset(v_sb[:, :, :, 32:33], 1.0)
            with tc.tile_pool(name="mm", bufs=2, space="PSUM") as mmp:
                for co in range(2):
                    for f in range(N // 512):
                        sl = slice(f * 512, (f + 1) * 512)
                        ps = mmp.tile([128, 512], f32, tag="proj", name="proj")
                        for ci in range(2):
                            nc.tensor.matmul(ps[:, :], w_sb["k", ci][:, co * 128:(co + 1) * 128],
                                             tn[ci][:, sl], start=(ci == 0), stop=(ci == 1))
                        nc.scalar.copy(kT[co][:, sl], ps[:, :])
                    for f in range(NQ // 512):
                        sl = slice(f * 512, (f + 1) * 512)
                        ps = mmp.tile([128, 512], f32, tag="proj", name="proj")
                        for ci in range(2):
                            nc.tensor.matmul(ps[:, :], w_sb["q", ci][:, co * 128:(co + 1) * 128],
                                             tn[ci][:, sl], start=(ci == 0), stop=(ci == 1))
                        nc.scalar.copy(qT[co][:, sl], ps[:, :])
                for j in range(32):
                    jl = slice(j * 128, (j + 1) * 128)
                    ps = mmp.tile([128, 256], f32, tag="vproj", name="vproj")
                    for ci in range(2):
                        nc.tensor.matmul(ps[:, :], tn[ci][:, jl], w_sb["v", ci][:, :],
                                         start=(ci == 0), stop=(ci == 1))
                    nc.vector.tensor_copy(v_sb[:, j, :, 0:32],
                                          ps[:, :].rearrange("p (h d) -> p h d", h=NH))

            # ---- attention ----
            attnT = [big.tile([128, NQ], bf16, tag=f"at{c}", name=f"at{c}") for c in range(2)]
            with tc.tile_pool(name="sps", bufs=1, space="PSUM") as sp, \
                 tc.tile_pool(name="avp", bufs=1, space="PSUM") as avp, \
                 tc.tile_pool(name="xtr", bufs=2, space="PSUM") as xtr, \
                 tc.tile_pool(name="pp", bufs=3) as ppool, \
                 tc.tile_pool(name="nrm", bufs=4) as nrm:
                for f in range(NQ // 512):
                    fl = slice(f * 512, (f + 1) * 512)
                    for hg in range(2):
                        av = [avp.tile([128, 512], f32, tag=f"av{pr}", name=f"av{pr}") for pr in range(2)]
                        for j in range(32):
                            jl = slice(j * 128, (j + 1) * 128)
                            ss = [sp.tile([128, 512], f32, tag=f"s{i}", name=f"s{i}") for i in range(4)]
                            pt = [ppool.tile([128, 512], bf16, tag=f"p{i}", name=f"p{i}") for i in range(4)]
                            for i in range(4):
                                rr = slice(i * 32, (i + 1) * 32)
                                nc.tensor.matmul(ss[i][:, :], kT[hg][rr, jl], qT[hg][rr, fl],
                                                 start=True, stop=True,
                                                 tile_position=(i * 32, 0))
                            for i in range(4):
                                if j % 8 < ACT_EXP_SHARE:
                                    nc.scalar.activation(pt[i][:, :], ss[i][:, :],
                                                         ACTF.Exp, scale=LN2 / 128.0)
                                else:
                                    nc.vector.tensor_scalar(
                                        pt[i][:, :].bitcast(i16), ss[i][:, :],
                                        B16F, None, ALU.add)
                            for pr in range(2):
                                for t2 in range(2):
                                    h = pr * 2 + t2
                                    nc.tensor.matmul(
                                        av[pr][t2 * 64:t2 * 64 + 33, :],
                                        v_sb[:, j, hg * 4 + h, :], pt[h][:, :],
                                        start=(j == 0), stop=(j == 31),
                                        tile_position=(0, t2 * 64))
                        for pr in range(2):
                            for t2 in range(2):
                                rbase = t2 * 64
                                rcp = nrm.tile([1, 512], f32, tag=f"rc{pr}{t2}", name=f"rc{pr}{t2}")
                                nc.vector.reciprocal(rcp[:, :], av[pr][rbase + 32:rbase + 33, :])
                                bc = xtr.tile([32, 512], f32, tag="bc", name="bc")
                                nc.tensor.matmul(bc[:, :], ones_row[:, 0:32], rcp[:, :],
                                                 start=True, stop=True)
                                bcs = nrm.tile([32, 512], f32, tag="bcs", name="bcs")
                                nc.vector.tensor_copy(bcs[:, :], bc[:, :])
                                row0 = (pr * 2 + t2) * 32
                                nc.vector.tensor_tensor(
                                    attnT[hg][row0:row0 + 32, fl],
                                    av[pr][rbase:rbase + 32, :], bcs[:, :], ALU.mult)

            # ---- output projection + residual ----
            with tc.tile_pool(name="mm2", bufs=2, space="PSUM") as mm2, \
                 tc.tile_pool(name="ot", bufs=4) as otp:
                for mo in range(2):
                    for f in range(NQ // 512):
                        sl = slice(f * 512, (f + 1) * 512)
                        ps = mm2.tile([128, 512], f32, tag="o", name="o")
                        for ci in range(2):
                            nc.tensor.matmul(ps[:, :], w_sb["p", ci][:, mo * 128:(mo + 1) * 128],
                                             attnT[ci][:, sl], start=(ci == 0), stop=(ci == 1))
                        ot = otp.tile([128, 512], f32, tag="ot", name="ot")
                        nc.vector.tensor_tensor(ot[:, :], ps[:, :], x_sb[mo][:, sl], ALU.add)
                        nc.vector.tensor_scalar(ot[:, :], ot[:, :], bp_sb[mo][:, :],
                                                None, ALU.add)
                        nc.sync.dma_start(out=od[mo * 128:(mo + 1) * 128, sl], in_=ot[:, :])

    nc.finalize()
    return nc


def kernel(x, ln_gamma, ln_beta, w_qkv, w_proj, b_proj):
    import ml_dtypes
    from concourse.bass_utils import run_bass_kernel_spmd

    if "nc" not in _CACHE:
        _CACHE["nc"] = _build()
    nc = _CACHE["nc"]

    x = np.asarray(x, np.float32)
    w_qkv = np.asarray(w_qkv, np.float32)
    bf = ml_dtypes.bfloat16
    wqT = np.ascontiguousarray((A_SCALE * w_qkv[0:C]).T.astype(bf))
    wkT = np.ascontiguousarray(w_qkv[C:2 * C].T.astype(bf))
    wvT = np.ascontiguousarray(w_qkv[2 * C:3 * C].T.astype(bf))
    wpT = np.ascontiguousarray(np.asarray(w_proj, np.float32).T.astype(bf))
    gam = np.asarray(ln_gamma, np.float32).reshape(C, 1)
    bet = np.asarray(ln_beta, np.float32).reshape(C, 1)
    bp = np.asarray(b_proj, np.float32).reshape(C, 1)

    xf = x.reshape(B, C, N)
    in_maps = []
    for core in range(8):
        b, qc = core // 4, core % 4
        xr = np.roll(xf[b], -qc * NQ, axis=1)
        in_maps.append({
            "x": np.ascontiguousarray(xr), "wqT": wqT, "wkT": wkT,
            "wvT": wvT, "wpT": wpT, "gam": gam, "bet": bet, "bp": bp,
        })

    res = run_bass_kernel_spmd(nc, in_maps, core_ids=list(range(8)),
                               trace=_PROFILE)
    if _PROFILE:
        _CACHE["exec_time_ns"] = res.exec_time_ns
    out = np.empty((B, C, N), np.float32)
    for core in range(8):
        b, qc = core // 4, core % 4
        out[b][:, qc * NQ:(qc + 1) * NQ] = res.results[core]["out"]
    return out.reshape(B, C, H, W)


# revision 13
# speedup vs baseline: 1.0023x; 1.0023x over previous
"""Trainium2 Bass kernel for nn_Attention (B=2, C=256, H=W=64, 8 heads).

Sharding: 8 cores = 2 batches x 4 query-chunks (1024 queries each), no
collectives. Each core gets its batch's full x with token columns rolled so
its own query chunk sits at columns 0:1024 (attention is permutation-
invariant over keys); it computes LN + projections + attention for its
queries and writes a [256, 1024] slice of the output.

Everything stays in the transposed [channel, token] layout (x's native DRAM
layout): LN (stats via ones-matmul over the partition dim) -> qT/kT
projections -> S^T = K Q^T (K=32 matmuls packed 4-way into PE row groups)
-> exp -> P^T V via stationary-V matmuls with an appended ones column
(softmax denominators for free) -> normalize -> out-projection -> residual.

exp is split between ScalarE (true exp) and VectorE (Schraudolph: Wq is
pre-scaled so PSUM = 128*log2(e)*logit; adding a magic bias and converting
f32->int16 yields the bf16 bit pattern of 2^y, ~3% max rel err, harmless
here because the attention branch contributes ~0.2% of the output next to
the residual).
"""

import numpy as np

B, C, H, W = 2, 256, 64, 64
N = H * W            # 4096 tokens
NH, HD = 8, 32       # heads, head_dim
NQ = N // 4          # queries per core
LN_EPS = 1e-5
LOG2E = 1.4426950408889634
LN2 = 0.6931471805599453
ATTN_SCALE = HD ** -0.5
A_SCALE = 128.0 * LOG2E * ATTN_SCALE   # folded into Wq on host
B16F = 16256.0 - 5.6                   # Schraudolph bias (calibrated)
ACT_EXP_SHARE = 5                      # j%8 < ACT_EXP_SHARE -> ScalarE exp

_PROFILE = False
_CACHE = {}


def _build():
    from concourse import bacc
    from concourse import mybir
    import concourse.tile as tile
    import dataclasses

    f32 = mybir.dt.float32
    bf16 = mybir.dt.bfloat16
    i16 = mybir.dt.int16
    ALU = mybir.AluOpType
    ACTF = mybir.ActivationFunctionType

    nc = bacc.Bacc("TRN2", target_bir_lowering=False)
    xd = nc.dram_tensor("x", [C, N], f32, kind="ExternalInput")
    wq = nc.dram_tensor("wqT", [C, C], bf16, kind="ExternalInput")  # pre-scaled
    wk = nc.dram_tensor("wkT", [C, C], bf16, kind="ExternalInput")
    wv = nc.dram_tensor("wvT", [C, C], bf16, kind="ExternalInput")
    wp = nc.dram_tensor("wpT", [C, C], bf16, kind="ExternalInput")
    gam = nc.dram_tensor("gam", [C, 1], f32, kind="ExternalInput")
    bet = nc.dram_tensor("bet", [C, 1], f32, kind="ExternalInput")
    bpd = nc.dram_tensor("bp", [C, 1], f32, kind="ExternalInput")
    od = nc.dram_tensor("out", [C, NQ], f32, kind="ExternalOutput")

    def bcast(ap, parts):
        # replicate one partition across `parts` partitions (DMA source only)
        return dataclasses.replace(ap, ap=[[0, parts]] + list(ap.ap[1:]))

    with tile.TileContext(nc) as tc:
        with tc.tile_pool(name="big", bufs=1) as big, \
             tc.tile_pool(name="sml", bufs=4) as sml:

            # ---- load inputs ----
            x_sb = [big.tile([128, N], f32, tag=f"x{c}", name=f"x{c}") for c in range(2)]
            for c in range(2):
                nc.sync.dma_start(out=x_sb[c][:, :], in_=xd[c * 128:(c + 1) * 128, :])
            w_sb = {}
            for name, t in (("q", wq), ("k", wk), ("v", wv), ("p", wp)):
                for c in range(2):
                    s = big.tile([128, C], bf16, tag=f"w{name}{c}", name=f"w{name}{c}")
                    nc.sync.dma_start(out=s[:, :], in_=t[c * 128:(c + 1) * 128, :])
                    w_sb[name, c] = s
            gam_sb = [big.tile([128, 1], f32, tag=f"g{c}", name=f"g{c}") for c in range(2)]
            bet_sb = [big.tile([128, 1], f32, tag=f"b{c}", name=f"b{c}") for c in range(2)]
            bp_sb = [big.tile([128, 1], f32, tag=f"bp{c}", name=f"bp{c}") for c in range(2)]
            for c in range(2):
                nc.sync.dma_start(out=gam_sb[c][:, :], in_=gam[c * 128:(c + 1) * 128, :])
                nc.sync.dma_start(out=bet_sb[c][:, :], in_=bet[c * 128:(c + 1) * 128, :])
                nc.sync.dma_start(out=bp_sb[c][:, :], in_=bpd[c * 128:(c + 1) * 128, :])
            ones_sb = big.tile([128, 1], f32, tag="ones", name="ones")
            nc.vector.memset(ones_sb[:, :], 1.0 / C)
            ones_row = big.tile([1, 128], f32, tag="onesr", name="onesr")
            nc.vector.memset(ones_row[:, :], 1.0)

            tn = [big.tile([128, N], bf16, tag=f"tn{c}", name=f"tn{c}") for c in range(2)]

            # ---- LayerNorm ----
            with tc.tile_pool(name="lnp", bufs=1) as lnp, \
                 tc.tile_pool(name="lns", bufs=2, space="PSUM") as lns:
                sq = [lnp.tile([128, N], f32, tag=f"sq{c}", name=f"sq{c}") for c in range(2)]
                for c in range(2):
                    nc.scalar.activation(sq[c][:, :], x_sb[c][:, :], ACTF.Square)
                eps_sc = lnp.tile([1, 1], f32, tag="epssc", name="epssc")
                nc.vector.memset(eps_sc[:, :], LN_EPS)
                for f in range(8):
                    sl = slice(f * 512, (f + 1) * 512)
                    mps = lns.tile([1, 512], f32, tag="mps", name="mps")
                    nc.tensor.matmul(mps[:, :], ones_sb[:, :], x_sb[0][:, sl], start=True, stop=False)
                    nc.tensor.matmul(mps[:, :], ones_sb[:, :], x_sb[1][:, sl], start=False, stop=True)
                    mu_sb = sml.tile([1, 512], f32, tag="musb", name="musb")
                    nc.vector.tensor_copy(mu_sb[:, :], mps[:, :])
                    eps_t = lns.tile([1, 512], f32, tag="eps", name="eps")
                    nc.tensor.matmul(eps_t[:, :], ones_sb[:, :], sq[0][:, sl], start=True, stop=False)
                    nc.tensor.matmul(eps_t[:, :], ones_sb[:, :], sq[1][:, sl], start=False, stop=True)
                    var_sb = sml.tile([1, 512], f32, tag="varsb", name="varsb")
                    nc.vector.tensor_tensor(var_sb[:, :], mu_sb[:, :], mu_sb[:, :], ALU.mult)
                    nc.vector.tensor_tensor(var_sb[:, :], eps_t[:, :], var_sb[:, :], ALU.subtract)
                    std_sb = sml.tile([1, 512], f32, tag="stdsb", name="stdsb")
                    nc.scalar.activation(std_sb[:, :], var_sb[:, :], ACTF.Sqrt, bias=eps_sc[:, :])
                    rs_sb = sml.tile([1, 512], f32, tag="rssb", name="rssb")
                    nc.vector.reciprocal(rs_sb[:, :], std_sb[:, :])
                    mu_b = lns.tile([128, 512], f32, tag="mub", name="mub")
                    rs_b = lns.tile([128, 512], f32, tag="rsb", name="rsb")
                    nc.tensor.matmul(mu_b[:, :], ones_row[:, :], mu_sb[:, :],
                                     start=True, stop=True, tile_position=(0, 0))
                    nc.tensor.matmul(rs_b[:, :], ones_row[:, :], rs_sb[:, :],
                                     start=True, stop=True, tile_position=(0, 0))
                    for c in range(2):
                        t = lnp.tile([128, 512], f32, tag=f"t{c}", name=f"t{c}")
                        nc.vector.tensor_tensor(t[:, :], x_sb[c][:, sl], mu_b[:, :], ALU.subtract)
                        nc.vector.tensor_tensor(t[:, :], t[:, :], rs_b[:, :], ALU.mult)
                        nc.vector.tensor_scalar(tn[c][:, sl], t[:, :], gam_sb[c][:, :],
                                                bet_sb[c][:, :], ALU.mult, ALU.add)

            # ---- q/k/v projections ----
            qT = [big.tile([128, NQ], bf16, tag=f"qT{c}", name=f"qT{c}") for c in range(2)]
            kT = [big.tile([128, N], bf16, tag=f"kT{c}", name=f"kT{c}") for c in range(2)]
            v_sb = big.tile([128, 32, NH, 33], bf16, tag="v", name="v")
            nc.vector.memset(v_sb[:, :, :, 32:33], 1.0)
            with tc.tile_pool(name="mm", bufs=2, space="PSUM") as mmp:
                for co in range(2):
                    for f in range(N // 512):
                        sl = slice(f * 512, (f + 1) * 512)
                        ps = mmp.tile([128, 512], f32, tag="proj", name="proj")
                        for ci in range(2):
                            nc.tensor.matmul(ps[:, :], w_sb["k", ci][:, co * 128:(co + 1) * 128],
                                             tn[ci][:, sl], start=(ci == 0), stop=(ci == 1))
                        nc.scalar.copy(kT[co][:, sl], ps[:, :])
                    for f in range(NQ // 512):
                        sl = slice(f * 512, (f + 1) * 512)
                        ps = mmp.tile([128, 512], f32, tag="proj", name="proj")
                        for ci in range(2):
                            nc.tensor.matmul(ps[:, :], w_sb["q", ci][:, co * 128:(co + 1) * 128],
                                             tn[ci][:, sl], start=(ci == 0), stop=(ci == 1))
                        nc.scalar.copy(qT[co][:, sl], ps[:, :])
                for j in range(32):
                    jl = slice(j * 128, (j + 1) * 128)
                    ps = mmp.tile([128, 256], f32, tag="vproj", name="vproj")
                    for ci in range(2):
                        nc.tensor.matmul(ps[:, :], tn[ci][:, jl], w_sb["v", ci][:, :],
                                         start=(ci == 0), stop=(ci == 1))
                    nc.vector.tensor_copy(v_sb[:, j, :, 0:32],
                                          ps[:, :].rearrange("p (h d) -> p h d", h=NH))

            # ---- attention ----
            attnT = [big.tile([128, NQ], bf16, tag=f"at{c}", name=f"at{c}") for c in range(2)]
            with tc.tile_pool(name="sps", bufs=2, space="PSUM") as sp, \
                 tc.tile_pool(name="avp", bufs=1, space="PSUM") as avp, \
                 tc.tile_pool(name="xtr", bufs=2, space="PSUM") as xtr, \
                 tc.tile_pool(name="pp", bufs=3) as ppool, \
                 tc.tile_pool(name="nrm", bufs=4) as nrm:
                for f in range(NQ // 512):
                    fl = slice(f * 512, (f + 1) * 512)
                    for hg in range(2):
                        av = [avp.tile([128, 512], f32, tag=f"av{pr}", name=f"av{pr}") for pr in range(2)]
                        for j in range(32):
                            jl = slice(j * 128, (j + 1) * 128)
                            ss = [sp.tile([128, 512], f32, tag=f"s{i % 2}", name=f"s{i % 2}") for i in range(4)]
                            pt = [ppool.tile([128, 512], bf16, tag=f"p{i}", name=f"p{i}") for i in range(4)]
                            for i in range(4):
                                rr = slice(i * 32, (i + 1) * 32)
                                nc.tensor.matmul(ss[i][:, :], kT[hg][rr, jl], qT[hg][rr, fl],
                                                 start=True, stop=True,
                                                 tile_position=(i * 32, 0))
                            for i in range(4):
                                if j % 8 < ACT_EXP_SHARE:
                                    nc.scalar.activation(pt[i][:, :], ss[i][:, :],
                                                         ACTF.Exp, scale=LN2 / 128.0)
                                else:
                                    nc.vector.tensor_scalar(
                                        pt[i][:, :].bitcast(i16), ss[i][:, :],
                                        B16F, None, ALU.add)
                            for pr in range(2):
                                for t2 in range(2):
                                    h = pr * 2 + t2
                                    nc.tensor.matmul(
                                        av[pr][t2 * 64:t2 * 64 + 33, :],
                                        v_sb[:, j, hg * 4 + h, :], pt[h][:, :],
                                        start=(j == 0), stop=(j == 31),
                                        tile_position=(0, t2 * 64))
                        for pr in range(2):
                            for t2 in range(2):
                                rbase = t2 * 64
                                rcp = nrm.tile([1, 512], f32, tag=f"rc{pr}{t2}", name=f"rc{pr}{t2}")
                                nc.vector.reciprocal(rcp[:, :], av[pr][rbase + 32:rbase + 33, :])
                                bc = xtr.tile([32, 512], f32, tag="bc", name="bc")
                                nc.tensor.matmul(bc[:, :], ones_row[:, 0:32], rcp[:, :],
                                                 start=True, stop=True)
                                bcs = nrm.tile([32, 512], f32, tag="bcs", name="bcs")
                                nc.vector.tensor_copy(bcs[:, :], bc[:, :])
                                row0 = (pr * 2 + t2) * 32
                                nc.vector.tensor_tensor(
                                    attnT[hg][row0:row0 + 32, fl],
                                    av[pr][rbase:rbase + 32, :], bcs[:, :], ALU.mult)

            # ---- output projection + residual ----
            with tc.tile_pool(name="mm2", bufs=2, space="PSUM") as mm2, \
                 tc.tile_pool(name="ot", bufs=4) as otp:
                for mo in range(2):
                    for f in range(NQ // 512):
                        sl = slice(f * 512, (f + 1) * 512)
                        ps = mm2.tile([128, 512], f32, tag="o", name="o")
                        for ci in range(2):
                            nc.tensor.matmul(ps[:, :], w_sb["p", ci][:, mo * 128:(mo + 1) * 128],
                                             attnT[ci][:, sl], start=(ci == 0), stop=(ci == 1))
                        ot = otp.tile([128, 512], f32, tag="ot", name="ot")
                        nc.vector.tensor_tensor(ot[:, :], ps[:, :], x_sb[mo][:, sl], ALU.add)
                        nc.vector.tensor_scalar(ot[:, :], ot[:, :], bp_sb[mo][:, :],
                                                None, ALU.add)
                        nc.sync.dma_start(out=od[mo * 128:(mo + 1) * 128, sl], in_=ot[:, :])

    nc.finalize()
    return nc


def kernel(x, ln_gamma, ln_beta, w_qkv, w_proj, b_proj):
    import ml_dtypes
    from concourse.bass_utils import run_bass_kernel_spmd

    if "nc" not in _CACHE:
        _CACHE["nc"] = _build()
    nc = _CACHE["nc"]

    x = np.asarray(x, np.float32)
    w_qkv = np.asarray(w_qkv, np.float32)
    bf = ml_dtypes.bfloat16
    wqT = np.ascontiguousarray((A_SCALE * w_qkv[0:C]).T.astype(bf))
    wkT = np.ascontiguousarray(w_qkv[C:2 * C].T.astype(bf))
    wvT = np.ascontiguousarray(w_qkv[2 * C:3 * C].T.astype(bf))
    wpT = np.ascontiguousarray(np.asarray(w_proj, np.float32).T.astype(bf))
    gam = np.asarray(ln_gamma, np.float32).reshape(C, 1)
    bet = np.asarray(ln_beta, np.float32).reshape(C, 1)
    bp = np.asarray(b_proj, np.float32).reshape(C, 1)

    xf = x.reshape(B, C, N)
    in_maps = []
    for core in range(8):
        b, qc = core // 4, core % 4
        xr = np.roll(xf[b], -qc * NQ, axis=1)
        in_maps.append({
            "x": np.ascontiguousarray(xr), "wqT": wqT, "wkT": wkT,
            "wvT": wvT, "wpT": wpT, "gam": gam, "bet": bet, "bp": bp,
        })

    res = run_bass_kernel_spmd(nc, in_maps, core_ids=list(range(8)),
                               trace=_PROFILE)
    if _PROFILE:
        _CACHE["exec_time_ns"] = res.exec_time_ns
    out = np.empty((B, C, N), np.float32)
    for core in range(8):
        b, qc = core // 4, core % 4
        out[b][:, qc * NQ:(qc + 1) * NQ] = res.results[core]["out"]
    return out.reshape(B, C, H, W)
